# revision 12
# baseline (speedup 1.0000x reference)
"""Bass/Tile kernel for nn_MultiHeadAttention (B=2, S=2048, D=1024, H=16).

Sharding: 8 cores = 2 (batch) x 4 (head-chunks of 4 heads).

v6 design (baseline v1 ~307us):
  - All matmul paths in BF16 (inputs, weights, qp/kp/vp/at/hc): bf16
    LDWEIGHTS (125ns) hide under N=512 streams; input DMA halves.
    Host-side full-datapath sim: rel err 0.0112 vs the 2e-2 gate.
  - kb-inner attention with PV accumulated in PSUM across all 16
    k-blocks (start/stop) -> no DVE accumulation chain.
  - PV matmuls software-pipelined one kb behind the score/exp stream so
    the in-order PE queue never head-of-line blocks on the z-chain.
  - PSUM: 4 banks pvt (2x[128,1024] fp32, col-packed heads) + 4 banks
    score rotation (2x[128,1024]).  K-proj chunks 1-3, V-proj tiles
    4-15 and pair-0's O-projection ride the score rotation as short
    MM-only bursts (all DMA prefetched into dedicated buffers).
  - O-projection split per head-pair into out0/out1 bf16 partials
    (summed on host with bo): pair-0's O-proj + DMA runs during
    pair-1's attention; only pair-1's is tail.
"""

import sys

sys.path.insert(0, "/opt/trn_rl_repo")

from contextlib import ExitStack

import numpy as np
import ml_dtypes

import concourse.bass as bass
import concourse.mybir as mybir
import concourse.tile as tile
from concourse import bacc
from concourse.bass_utils import run_bass_kernel_spmd

BF16 = mybir.dt.bfloat16
F32 = mybir.dt.float32
AF = mybir.ActivationFunctionType
ALU = mybir.AluOpType
I16 = mybir.dt.int16

# Schraudolph fast-exp in bf16 bit space: bf16(2^x) bits ~ int16(128*x + B)
FEXP_KBS = (3, 8, 13)
FEXP_A = 128.0 / float(np.log(2.0))
FEXP_B = 127.0 * 128.0 - 366392.0 / 65536.0

D = 1024
NK = 8  # k-tiles over D
DOUT = 256  # per-core head dims (4 heads)
NPAIR = 2  # pairs of heads (128 dout each)
HD = 64
S = 2048
NKB = S // 128  # 16 k-token blocks
NQH = 2  # q halves of 1024
NTC = S // 512  # 4 projection token chunks
NTT = S // 128  # 16 token tiles


def build_kernel():
    nc = bacc.Bacc("TRN2", target_bir_lowering=False, debug=False)

    qT = nc.dram_tensor("qT", [NTC, 128, NK, 512], BF16, kind="ExternalInput")
    kT = nc.dram_tensor("kT", [NTC, 128, NK, 512], BF16, kind="ExternalInput")
    vT = nc.dram_tensor("vT", [4, 128, NK, 512], BF16, kind="ExternalInput")
    wq = nc.dram_tensor("wq", [D, DOUT], BF16, kind="ExternalInput")
    wk = nc.dram_tensor("wk", [D, DOUT], BF16, kind="ExternalInput")
    wv = nc.dram_tensor("wv", [D, DOUT], BF16, kind="ExternalInput")
    wo = nc.dram_tensor("wo", [DOUT, D], BF16, kind="ExternalInput")
    bq = nc.dram_tensor("bq", [NPAIR, 128, 1], F32, kind="ExternalInput")
    bk = nc.dram_tensor("bk", [NPAIR, 128, 1], F32, kind="ExternalInput")
    bv = nc.dram_tensor("bv", [DOUT], F32, kind="ExternalInput")
    out0 = nc.dram_tensor("out0", [S, D], BF16, kind="ExternalOutput")
    out1 = nc.dram_tensor("out1", [S, D], BF16, kind="ExternalOutput")

    qTv = qT.ap()  # [NTC, 128, NK, 512] chunk-major, 8KB/partition contig
    kTv = kT.ap()
    vTv = vT.ap()  # [4, 128, NK, 512] quad-major
    wqv = wq.ap().rearrange("(t p) m -> p t m", p=128)  # [128, 8, 256]
    wkv = wk.ap().rearrange("(t p) m -> p t m", p=128)
    wvv = wv.ap().rearrange("(t p) m -> p t m", p=128)
    wov = wo.ap().rearrange("(t p) m -> p t m", p=128)  # [128, 2, 1024]
    bqv = bq.ap().rearrange("a p o -> p a o")  # [128, 2, 1]
    bkv = bk.ap().rearrange("a p o -> p a o")
    outv = [
        out0.ap().rearrange("(tb two p) m -> tb p two m", two=2, p=128),
        out1.ap().rearrange("(tb two p) m -> tb p two m", two=2, p=128),
    ]  # [8, 128, 2, 1024]

    bv_bcast_ap = bass.AP(tensor=bv.ap().tensor, offset=0, ap=[[0, 128], [1, DOUT]])

    with tile.TileContext(nc) as tc, ExitStack() as ctx:
        sb = ctx.enter_context(tc.tile_pool(name="sb", bufs=1))
        xtp = ctx.enter_context(tc.tile_pool(name="xt_stream", bufs=1))
        vtp = ctx.enter_context(tc.tile_pool(name="vt_stream", bufs=1))
        atp = ctx.enter_context(tc.tile_pool(name="at_pool", bufs=1))
        zp = ctx.enter_context(tc.tile_pool(name="z_pool", bufs=1))
        osb = ctx.enter_context(tc.tile_pool(name="o_sb", bufs=1))
        psa = ctx.enter_context(tc.tile_pool(name="ps_all", bufs=1, space="PSUM"))

        # ---- resident weights/biases (all small, bf16) ----
        wq_sb = sb.tile([128, NK, DOUT], BF16, tag="wq")
        wk_sb = sb.tile([128, NK, DOUT], BF16, tag="wk")
        wv_sb = sb.tile([128, NK, DOUT], BF16, tag="wv")
        wo_sb = sb.tile([128, NPAIR, D], BF16, tag="wo")
        bq_sb = sb.tile([128, NPAIR, 1], F32, tag="bq")
        bk_sb = sb.tile([128, NPAIR, 1], F32, tag="bk")
        bv_sb = sb.tile([128, DOUT], F32, tag="bv")
        nc.sync.dma_start(out=wq_sb[:], in_=wqv)
        nc.sync.dma_start(out=wk_sb[:], in_=wkv)
        nc.sync.dma_start(out=wv_sb[:], in_=wvv)
        nc.sync.dma_start(out=bq_sb[:], in_=bqv)
        nc.sync.dma_start(out=bk_sb[:], in_=bkv)
        nc.sync.dma_start(out=bv_sb[:], in_=bv_bcast_ap)

        # ---- projection outputs (resident bf16) ----
        qpT_sb = sb.tile([128, NPAIR, S], BF16, tag="qpT")
        kpT_sb = sb.tile([128, NPAIR, S], BF16, tag="kpT")
        vp_sb = sb.tile([128, NTT, DOUT], BF16, tag="vp")
        hcT_sb = sb.tile([128, NPAIR, S], BF16, tag="hcT")

        def qkproj_dma(XTv, tci):
            xt = xtp.tile([128, NK, 512], BF16, tag="xt", bufs=5, name="xt")
            nc.sync.dma_start(out=xt[:], in_=XTv[tci])
            return xt

        def qkproj_mms(xts, W_sb, b_sb, XPT, tci, pairs):
            tsl = slice(tci * 512, tci * 512 + 512)
            ps = psa.tile([128, 1024], F32, tag="sc", bufs=2, name="pj")
            for kk in range(NK):
                for p in pairs:
                    nc.tensor.matmul(
                        ps[:, p * 512 : p * 512 + 512],
                        lhsT=W_sb[:, kk, p * 128 : p * 128 + 128],
                        rhs=xts[:, kk, :],
                        start=(kk == 0),
                        stop=(kk == NK - 1),
                        skip_group_check=True,
                    )
            for p in pairs:
                nc.vector.tensor_scalar_add(
                    XPT[:, p, tsl], ps[:, p * 512 : p * 512 + 512], b_sb[:, p, :]
                )

        def qkproj_chunk(XTv, W_sb, b_sb, XPT, tci):
            qkproj_mms(qkproj_dma(XTv, tci), W_sb, b_sb, XPT, tci, (0, 1))

        def vproj_dma(tq):
            vt = vtp.tile([128, NK, 512], BF16, tag="vt", bufs=4, name="vt")
            nc.gpsimd.dma_start(out=vt[:], in_=vTv[tq])
            return vt

        def vproj_mms(vts, tq, idxs):
            # V projection for the given tile indices within quad tq
            ps = psa.tile([128, 1024], F32, tag="sc", bufs=2, name="vj")
            for j, i in enumerate(idxs):
                for kk in range(NK):
                    nc.tensor.matmul(
                        ps[:, j * 256 : j * 256 + 256],
                        lhsT=vts[:, kk, i * 128 : i * 128 + 128],
                        rhs=wv_sb[:, kk, :],
                        start=(kk == 0),
                        stop=(kk == NK - 1),
                        skip_group_check=True,
                    )
            for j, i in enumerate(idxs):
                nc.vector.scalar_tensor_tensor(
                    out=vp_sb[:, tq * 4 + i, :],
                    in0=ps[:, j * 256 : j * 256 + 256],
                    scalar=1.0,
                    in1=bv_sb[:],
                    op0=ALU.mult,
                    op1=ALU.add,
                )

        ost_cur = [None]

        def oproj_tt(p, tt, on_scalar=False):
            # fills half of a [128,2048] staging tile; DMA fires per tt pair
            ps = psa.tile([128, 1024], F32, tag="sc", bufs=2, name="oj")
            for dc in range(2):
                nc.tensor.matmul(
                    ps[:, dc * 512 : dc * 512 + 512],
                    lhsT=hcT_sb[:, p, tt * 128 : tt * 128 + 128],
                    rhs=wo_sb[:, p, dc * 512 : dc * 512 + 512],
                    start=True,
                    stop=True,
                    skip_group_check=True,
                )
            if tt % 2 == 0:
                ost_cur[0] = osb.tile(
                    [128, 2, 1024], BF16, tag="ost", bufs=3, name="ost"
                )
            ost = ost_cur[0]
            half = ost[:, tt % 2, :]
            if on_scalar:
                nc.scalar.copy(half, ps[:])
            else:
                nc.vector.tensor_copy(half, ps[:])
            if tt % 2 == 1:
                nc.gpsimd.dma_start(out=outv[p][tt // 2], in_=ost[:])

        def attn_scores(p, kb):
            # scores + exp + z-chain for one k-block; returns PV operands
            ksl = slice(kb * 128, kb * 128 + 128)
            at_t = {}
            z_t = {}
            for qh in range(NQH):
                scs = [
                    psa.tile([128, 1024], F32, tag="sc", bufs=2, name=f"sc{h}")
                    for h in range(2)
                ]
                for qq in range(2):
                    for h in range(2):
                        hsl = slice(h * 64, h * 64 + 64)
                        qsl = slice(
                            qh * 1024 + qq * 512, qh * 1024 + qq * 512 + 512
                        )
                        nc.tensor.matmul(
                            scs[h][:, qq * 512 : qq * 512 + 512],
                            lhsT=kpT_sb[hsl, p, ksl],
                            rhs=qpT_sb[hsl, p, qsl],
                            start=True,
                            stop=True,
                            skip_group_check=True,
                        )
                for h in range(2):
                    z = zp.tile([128, 1], F32, tag=f"z{h}{qh}", bufs=6, name="z")
                    if kb in FEXP_KBS:
                        ati = atp.tile(
                            [128, 1024], I16, tag=f"atf{h}{qh}", bufs=2,
                            name="atf",
                        )
                        nc.vector.tensor_scalar(
                            out=ati[:], in0=scs[h][:],
                            scalar1=FEXP_A, scalar2=FEXP_B,
                            op0=ALU.mult, op1=ALU.add,
                        )
                        at_ap = ati[:].bitcast(BF16)
                        nc.vector.tensor_reduce(
                            out=z[:], in_=at_ap,
                            axis=mybir.AxisListType.X, op=ALU.add,
                        )
                        at_t[(h, qh)] = at_ap
                    else:
                        at = atp.tile(
                            [128, 1024], BF16, tag=f"at{h}{qh}", bufs=6,
                            name="at",
                        )
                        nc.scalar.activation(
                            out=at[:], in_=scs[h][:], func=AF.Exp, accum_out=z[:]
                        )
                        at_t[(h, qh)] = at
                    z_t[(h, qh)] = z
            vhss = {}
            for h in range(2):
                zs = zp.tile([128, 1], F32, tag=f"zs{h}", bufs=4, name="zs")
                nc.vector.tensor_add(zs[:], z_t[(h, 0)][:], z_t[(h, 1)][:])
                rz = zp.tile([128, 1], F32, tag=f"rz{h}", bufs=4, name="rz")
                nc.vector.reciprocal(rz[:], zs[:])
                vhs = zp.tile([128, HD], BF16, tag=f"vh{h}", bufs=4, name="vhs")
                nc.vector.tensor_scalar_mul(
                    vhs[:],
                    vp_sb[:, kb, p * 128 + h * 64 : p * 128 + h * 64 + 64],
                    rz[:],
                )
                vhss[h] = vhs
            return (kb, at_t, vhss)

        def attn_pv(pvt, state):
            # PV matmuls for a previous k-block (h-adjacent: col groups
            # run concurrently); accumulates into resident pvt tiles
            kb, at_t, vhss = state
            for qh in range(NQH):
                for qq in range(2):
                    for h in range(2):
                        nc.tensor.matmul(
                            pvt[qh][
                                h * 64 : h * 64 + 64, qq * 512 : qq * 512 + 512
                            ],
                            lhsT=vhss[h][:],
                            rhs=at_t[(h, qh)][:, qq * 512 : qq * 512 + 512],
                            start=(kb == 0),
                            stop=(kb == NKB - 1),
                            tile_position=(0, h * 64),
                            skip_group_check=True,
                        )

        # ---- pre-phase: Q-proj (all), K-proj chunk 0, V-proj quad 0 ----
        for tci in range(NTC):
            qkproj_chunk(qTv, wq_sb, bq_sb, qpT_sb, tci)
        qkproj_chunk(kTv, wk_sb, bk_sb, kpT_sb, 0)
        vq0 = vproj_dma(0)
        vq1 = vproj_dma(1)
        vq2 = vproj_dma(2)
        vq3 = vproj_dma(3)
        vproj_mms(vq0, 0, (0, 1))
        vproj_mms(vq0, 0, (2, 3))
        vproj_mms(vq1, 1, (0, 1))
        vproj_mms(vq1, 1, (2, 3))
        nc.sync.dma_start(out=wo_sb[:], in_=wov)
        kc1 = qkproj_dma(kTv, 1)
        kc2 = qkproj_dma(kTv, 2)

        # ---- attention pair 0 with inserted projection bursts ----
        pvt0 = [
            psa.tile([128, 1024], F32, tag="pvt", bufs=2, name=f"pvt0_{qh}")
            for qh in range(NQH)
        ]
        kc3 = None
        prev = None
        for kb in range(NKB):
            cur = attn_scores(0, kb)
            if prev is not None:
                attn_pv(pvt0, prev)
            prev = cur
            if kb == 0:
                qkproj_mms(kc1, wk_sb, bk_sb, kpT_sb, 1, (0,))
            elif kb == 1:
                qkproj_mms(kc1, wk_sb, bk_sb, kpT_sb, 1, (1,))
            elif kb == 2:
                vproj_mms(vq2, 2, (0, 1))
                kc3 = qkproj_dma(kTv, 3)
            elif kb == 3:
                vproj_mms(vq2, 2, (2, 3))
            elif kb == 4:
                qkproj_mms(kc2, wk_sb, bk_sb, kpT_sb, 2, (0,))
            elif kb == 5:
                qkproj_mms(kc2, wk_sb, bk_sb, kpT_sb, 2, (1,))
            elif kb == 6:
                vproj_mms(vq3, 3, (0, 1))
            elif kb == 7:
                vproj_mms(vq3, 3, (2, 3))
            elif kb == 8:
                qkproj_mms(kc3, wk_sb, bk_sb, kpT_sb, 3, (0,))
            elif kb == 9:
                qkproj_mms(kc3, wk_sb, bk_sb, kpT_sb, 3, (1,))
        attn_pv(pvt0, prev)
        for qh in range(NQH):
            nc.vector.tensor_copy(
                hcT_sb[:, 0, qh * 1024 : qh * 1024 + 1024], pvt0[qh][:]
            )

        # ---- attention pair 1; pair-0 O-projection rides along ----
        pvt1 = [
            psa.tile([128, 1024], F32, tag="pvt", bufs=2, name=f"pvt1_{qh}")
            for qh in range(NQH)
        ]
        prev = None
        for kb in range(NKB):
            cur = attn_scores(1, kb)
            if prev is not None:
                attn_pv(pvt1, prev)
            prev = cur
            oproj_tt(0, kb)
        attn_pv(pvt1, prev)
        for qh in range(NQH):
            nc.scalar.copy(
                hcT_sb[:, 1, qh * 1024 : qh * 1024 + 1024], pvt1[qh][:]
            )

        # ---- pair-1 O-projection tail (alternate drain engines) ----
        for tt in range(NTT):
            oproj_tt(1, tt, on_scalar=(tt % 2 == 0))

    nc.compile()
    return nc


# ---------------- host-side shard / unshard ----------------

B = 2

_NC_CACHE = {}


def _get_nc():
    if "nc" not in _NC_CACHE:
        _NC_CACHE["nc"] = build_kernel()
    return _NC_CACHE["nc"]


def make_in_maps(q, k, v, Wq, bq, Wk, bk, Wv, bv, Wo, bo):
    bf = ml_dtypes.bfloat16
    maps = []
    for c in range(8):
        b = c // 4
        hc = c % 4
        cols = slice(256 * hc, 256 * hc + 256)
        vt = (
            v[b]
            .astype(bf)
            .reshape(4, 512, NK, 128)
            .transpose(0, 3, 2, 1)
        )  # [quad, p(d-in-chunk), kk, t]
        maps.append({
            "qT": np.ascontiguousarray(
                q[b].astype(bf).reshape(NTC, 512, NK, 128).transpose(0, 3, 2, 1)
            ),
            "kT": np.ascontiguousarray(
                k[b].astype(bf).reshape(NTC, 512, NK, 128).transpose(0, 3, 2, 1)
            ),
            "vT": np.ascontiguousarray(vt),
            "wq": np.ascontiguousarray(Wq[:, cols].astype(bf)),
            "wk": np.ascontiguousarray(Wk[:, cols].astype(bf)),
            "wv": np.ascontiguousarray(Wv[:, cols].astype(bf)),
            "wo": np.ascontiguousarray(Wo[cols, :].astype(bf)),
            "bq": np.ascontiguousarray(
                bq[cols].reshape(NPAIR, 128, 1).astype(np.float32)
            ),
            "bk": np.ascontiguousarray(
                bk[cols].reshape(NPAIR, 128, 1).astype(np.float32)
            ),
            "bv": np.ascontiguousarray(bv[cols].astype(np.float32)),
        })
    return maps


def kernel(q, k, v, Wq, bq, Wk, bk, Wv, bv, Wo, bo):
    q = np.asarray(q, dtype=np.float32)
    k = np.asarray(k, dtype=np.float32)
    v = np.asarray(v, dtype=np.float32)
    Wq = np.asarray(Wq, dtype=np.float32)
    Wk = np.asarray(Wk, dtype=np.float32)
    Wv = np.asarray(Wv, dtype=np.float32)
    Wo = np.asarray(Wo, dtype=np.float32)
    bq = np.asarray(bq, dtype=np.float32)
    bk = np.asarray(bk, dtype=np.float32)
    bv = np.asarray(bv, dtype=np.float32)
    bo = np.asarray(bo, dtype=np.float32)

    nc = _get_nc()
    maps = make_in_maps(q, k, v, Wq, bq, Wk, bk, Wv, bv, Wo, bo)
    res = run_bass_kernel_spmd(nc, maps, core_ids=list(range(8)))

    outs = []
    for b in range(B):
        acc = np.zeros((S, D), dtype=np.float32)
        for hc in range(4):
            r = res.results[b * 4 + hc]
            acc += r["out0"].astype(np.float32)
            acc += r["out1"].astype(np.float32)
        acc += bo[None, :]
        outs.append(acc)
    return np.stack(outs, axis=0)


# revision 13
# speedup vs baseline: 1.0543x; 1.0543x over previous
"""Bass/Tile kernel for nn_MultiHeadAttention (B=2, S=2048, D=1024, H=16).

Sharding: 8 cores = 2 (batch) x 4 (head-chunks of 4 heads).

v6 design (baseline v1 ~307us):
  - All matmul paths in BF16 (inputs, weights, qp/kp/vp/at/hc): bf16
    LDWEIGHTS (125ns) hide under N=512 streams; input DMA halves.
    Host-side full-datapath sim: rel err 0.0112 vs the 2e-2 gate.
  - kb-inner attention with PV accumulated in PSUM across all 16
    k-blocks (start/stop) -> no DVE accumulation chain.
  - PV matmuls software-pipelined one kb behind the score/exp stream so
    the in-order PE queue never head-of-line blocks on the z-chain.
  - PSUM: 4 banks pvt (2x[128,1024] fp32, col-packed heads) + 4 banks
    score rotation (2x[128,1024]).  K-proj chunks 1-3, V-proj tiles
    4-15 and pair-0's O-projection ride the score rotation as short
    MM-only bursts (all DMA prefetched into dedicated buffers).
  - O-projection split per head-pair into out0/out1 bf16 partials
    (summed on host with bo): pair-0's O-proj + DMA runs during
    pair-1's attention; only pair-1's is tail.
"""

import sys

sys.path.insert(0, "/opt/trn_rl_repo")

from contextlib import ExitStack

import numpy as np
import ml_dtypes

import concourse.bass as bass
import concourse.mybir as mybir
import concourse.tile as tile
from concourse import bacc
from concourse.bass_utils import run_bass_kernel_spmd

BF16 = mybir.dt.bfloat16
F32 = mybir.dt.float32
AF = mybir.ActivationFunctionType
ALU = mybir.AluOpType
I16 = mybir.dt.int16

# Schraudolph fast-exp in bf16 bit space: bf16(2^x) bits ~ int16(128*x + B)
FEXP_KBS = ()  # unused
FEXP_A = 128.0 / float(np.log(2.0))
FEXP_B = 127.0 * 128.0 - 366392.0 / 65536.0

D = 1024
NK = 8  # k-tiles over D
DOUT = 256  # per-core head dims (4 heads)
NPAIR = 2  # pairs of heads (128 dout each)
HD = 64
S = 2048
NKB = S // 128  # 16 k-token blocks
NQH = 2  # q halves of 1024
NTC = S // 512  # 4 projection token chunks
NTT = S // 128  # 16 token tiles


def build_kernel():
    nc = bacc.Bacc("TRN2", target_bir_lowering=False, debug=False)

    qT = nc.dram_tensor("qT", [NTC, 128, NK, 512], BF16, kind="ExternalInput")
    kT = nc.dram_tensor("kT", [NTC, 128, NK, 512], BF16, kind="ExternalInput")
    vT = nc.dram_tensor("vT", [4, 128, NK, 512], BF16, kind="ExternalInput")
    wq = nc.dram_tensor("wq", [D, DOUT], BF16, kind="ExternalInput")
    wk = nc.dram_tensor("wk", [D, DOUT], BF16, kind="ExternalInput")
    wv = nc.dram_tensor("wv", [D, DOUT], BF16, kind="ExternalInput")
    wo = nc.dram_tensor("wo", [DOUT, D], BF16, kind="ExternalInput")
    bq = nc.dram_tensor("bq", [NPAIR, 128, 1], F32, kind="ExternalInput")
    bk = nc.dram_tensor("bk", [NPAIR, 128, 1], F32, kind="ExternalInput")
    bv = nc.dram_tensor("bv", [DOUT], F32, kind="ExternalInput")
    out0 = nc.dram_tensor("out0", [S, D], BF16, kind="ExternalOutput")
    out1 = nc.dram_tensor("out1", [S, D], BF16, kind="ExternalOutput")

    qTv = qT.ap()  # [NTC, 128, NK, 512] chunk-major, 8KB/partition contig
    kTv = kT.ap()
    vTv = vT.ap()  # [4, 128, NK, 512] quad-major
    wqv = wq.ap().rearrange("(t p) m -> p t m", p=128)  # [128, 8, 256]
    wkv = wk.ap().rearrange("(t p) m -> p t m", p=128)
    wvv = wv.ap().rearrange("(t p) m -> p t m", p=128)
    wov = wo.ap().rearrange("(t p) m -> p t m", p=128)  # [128, 2, 1024]
    bqv = bq.ap().rearrange("a p o -> p a o")  # [128, 2, 1]
    bkv = bk.ap().rearrange("a p o -> p a o")
    outv = [
        out0.ap().rearrange("(tb two p) m -> tb p two m", two=2, p=128),
        out1.ap().rearrange("(tb two p) m -> tb p two m", two=2, p=128),
    ]  # [8, 128, 2, 1024]

    bv_bcast_ap = bass.AP(tensor=bv.ap().tensor, offset=0, ap=[[0, 128], [1, DOUT]])

    with tile.TileContext(nc) as tc, ExitStack() as ctx:
        sb = ctx.enter_context(tc.tile_pool(name="sb", bufs=1))
        xtp = ctx.enter_context(tc.tile_pool(name="xt_stream", bufs=1))
        vtp = ctx.enter_context(tc.tile_pool(name="vt_stream", bufs=1))
        atp = ctx.enter_context(tc.tile_pool(name="at_pool", bufs=1))
        zp = ctx.enter_context(tc.tile_pool(name="z_pool", bufs=1))
        osb = ctx.enter_context(tc.tile_pool(name="o_sb", bufs=1))
        psa = ctx.enter_context(tc.tile_pool(name="ps_all", bufs=1, space="PSUM"))

        # ---- resident weights/biases (all small, bf16) ----
        wq_sb = sb.tile([128, NK, DOUT], BF16, tag="wq")
        wk_sb = sb.tile([128, NK, DOUT], BF16, tag="wk")
        wv_sb = sb.tile([128, NK, DOUT], BF16, tag="wv")
        wo_sb = sb.tile([128, NPAIR, D], BF16, tag="wo")
        bq_sb = sb.tile([128, NPAIR, 1], F32, tag="bq")
        bk_sb = sb.tile([128, NPAIR, 1], F32, tag="bk")
        bv_sb = sb.tile([128, DOUT], F32, tag="bv")
        nc.sync.dma_start(out=wq_sb[:], in_=wqv)
        nc.sync.dma_start(out=wk_sb[:], in_=wkv)
        nc.sync.dma_start(out=wv_sb[:], in_=wvv)
        nc.sync.dma_start(out=bq_sb[:], in_=bqv)
        nc.sync.dma_start(out=bk_sb[:], in_=bkv)
        nc.sync.dma_start(out=bv_sb[:], in_=bv_bcast_ap)

        # ---- projection outputs (resident bf16) ----
        qpT_sb = sb.tile([128, NPAIR, S], BF16, tag="qpT")
        kpT_sb = sb.tile([128, NPAIR, S], BF16, tag="kpT")
        vp_sb = sb.tile([128, NTT, DOUT], BF16, tag="vp")
        hcT_sb = sb.tile([128, NPAIR, S], BF16, tag="hcT")

        def qkproj_dma(XTv, tci):
            xt = xtp.tile([128, NK, 512], BF16, tag="xt", bufs=5, name="xt")
            nc.sync.dma_start(out=xt[:], in_=XTv[tci])
            return xt

        def qkproj_mms(xts, W_sb, b_sb, XPT, tci, pairs):
            tsl = slice(tci * 512, tci * 512 + 512)
            ps = psa.tile([128, 1024], F32, tag="sc", bufs=2, name="pj")
            for kk in range(NK):
                for p in pairs:
                    nc.tensor.matmul(
                        ps[:, p * 512 : p * 512 + 512],
                        lhsT=W_sb[:, kk, p * 128 : p * 128 + 128],
                        rhs=xts[:, kk, :],
                        start=(kk == 0),
                        stop=(kk == NK - 1),
                        skip_group_check=True,
                    )
            for p in pairs:
                nc.vector.tensor_scalar_add(
                    XPT[:, p, tsl], ps[:, p * 512 : p * 512 + 512], b_sb[:, p, :]
                )

        def qkproj_chunk(XTv, W_sb, b_sb, XPT, tci):
            qkproj_mms(qkproj_dma(XTv, tci), W_sb, b_sb, XPT, tci, (0, 1))

        def vproj_dma(tq):
            vt = vtp.tile([128, NK, 512], BF16, tag="vt", bufs=4, name="vt")
            nc.sync.dma_start(out=vt[:], in_=vTv[tq])
            return vt

        def vproj_mms(vts, tq, idxs):
            # V projection for the given tile indices within quad tq
            ps = psa.tile([128, 1024], F32, tag="sc", bufs=2, name="vj")
            for j, i in enumerate(idxs):
                for kk in range(NK):
                    nc.tensor.matmul(
                        ps[:, j * 256 : j * 256 + 256],
                        lhsT=vts[:, kk, i * 128 : i * 128 + 128],
                        rhs=wv_sb[:, kk, :],
                        start=(kk == 0),
                        stop=(kk == NK - 1),
                        skip_group_check=True,
                    )
            for j, i in enumerate(idxs):
                nc.vector.scalar_tensor_tensor(
                    out=vp_sb[:, tq * 4 + i, :],
                    in0=ps[:, j * 256 : j * 256 + 256],
                    scalar=1.0,
                    in1=bv_sb[:],
                    op0=ALU.mult,
                    op1=ALU.add,
                )

        ost_cur = [None]

        def oproj_tt(p, tt, on_scalar=False):
            # fills half of a [128,2048] staging tile; DMA fires per tt pair
            ps = psa.tile([128, 1024], F32, tag="sc", bufs=2, name="oj")
            for dc in range(2):
                nc.tensor.matmul(
                    ps[:, dc * 512 : dc * 512 + 512],
                    lhsT=hcT_sb[:, p, tt * 128 : tt * 128 + 128],
                    rhs=wo_sb[:, p, dc * 512 : dc * 512 + 512],
                    start=True,
                    stop=True,
                    skip_group_check=True,
                )
            if tt % 2 == 0:
                ost_cur[0] = osb.tile(
                    [128, 2, 1024], BF16, tag="ost", bufs=3, name="ost"
                )
            ost = ost_cur[0]
            half = ost[:, tt % 2, :]
            if on_scalar:
                nc.scalar.copy(half, ps[:])
            else:
                nc.vector.tensor_copy(half, ps[:])
            if tt % 2 == 1:
                nc.gpsimd.dma_start(out=outv[p][tt // 2], in_=ost[:])

        def attn_scores(p, kb):
            # scores + exp + z-chain for one k-block; returns PV operands
            ksl = slice(kb * 128, kb * 128 + 128)
            at_t = {}
            z_t = {}
            for qh in range(NQH):
                scs = [
                    psa.tile([128, 1024], F32, tag="sc", bufs=2, name=f"sc{h}")
                    for h in range(2)
                ]
                for qq in range(2):
                    for h in range(2):
                        hsl = slice(h * 64, h * 64 + 64)
                        qsl = slice(
                            qh * 1024 + qq * 512, qh * 1024 + qq * 512 + 512
                        )
                        nc.tensor.matmul(
                            scs[h][:, qq * 512 : qq * 512 + 512],
                            lhsT=kpT_sb[hsl, p, ksl],
                            rhs=qpT_sb[hsl, p, qsl],
                            start=True,
                            stop=True,
                            skip_group_check=True,
                        )
                for h in range(2):
                    z = zp.tile([128, 1], F32, tag=f"z{h}{qh}", bufs=6, name="z")
                    if h == 1 and qh == 1:
                        ati = atp.tile(
                            [128, 1024], I16, tag=f"atf{h}{qh}", bufs=4,
                            name="atf",
                        )
                        nc.vector.tensor_scalar(
                            out=ati[:], in0=scs[h][:],
                            scalar1=FEXP_A, scalar2=FEXP_B,
                            op0=ALU.mult, op1=ALU.add,
                        )
                        at_ap = ati[:].bitcast(BF16)
                        nc.vector.tensor_reduce(
                            out=z[:], in_=at_ap,
                            axis=mybir.AxisListType.X, op=ALU.add,
                        )
                        at_t[(h, qh)] = at_ap
                    else:
                        at = atp.tile(
                            [128, 1024], BF16, tag=f"at{h}{qh}", bufs=6,
                            name="at",
                        )
                        nc.scalar.activation(
                            out=at[:], in_=scs[h][:], func=AF.Exp, accum_out=z[:]
                        )
                        at_t[(h, qh)] = at
                    z_t[(h, qh)] = z
            vhss = {}
            for h in range(2):
                zs = zp.tile([128, 1], F32, tag=f"zs{h}", bufs=4, name="zs")
                nc.vector.tensor_add(zs[:], z_t[(h, 0)][:], z_t[(h, 1)][:])
                rz = zp.tile([128, 1], F32, tag=f"rz{h}", bufs=4, name="rz")
                nc.vector.reciprocal(rz[:], zs[:])
                vhs = zp.tile([128, HD], BF16, tag=f"vh{h}", bufs=4, name="vhs")
                nc.vector.tensor_scalar_mul(
                    vhs[:],
                    vp_sb[:, kb, p * 128 + h * 64 : p * 128 + h * 64 + 64],
                    rz[:],
                )
                vhss[h] = vhs
            return (kb, at_t, vhss)

        def attn_pv(pvt, state):
            # PV matmuls for a previous k-block (h-adjacent: col groups
            # run concurrently); accumulates into resident pvt tiles
            kb, at_t, vhss = state
            for qh in range(NQH):
                for qq in range(2):
                    for h in range(2):
                        nc.tensor.matmul(
                            pvt[qh][
                                h * 64 : h * 64 + 64, qq * 512 : qq * 512 + 512
                            ],
                            lhsT=vhss[h][:],
                            rhs=at_t[(h, qh)][:, qq * 512 : qq * 512 + 512],
                            start=(kb == 0),
                            stop=(kb == NKB - 1),
                            tile_position=(0, h * 64),
                            skip_group_check=True,
                        )

        # ---- pre-phase: Q-proj (all), K-proj chunk 0, V-proj quad 0 ----
        for tci in range(NTC):
            qkproj_chunk(qTv, wq_sb, bq_sb, qpT_sb, tci)
        qkproj_chunk(kTv, wk_sb, bk_sb, kpT_sb, 0)
        vq0 = vproj_dma(0)
        vq1 = vproj_dma(1)
        vproj_mms(vq0, 0, (0, 1))
        vproj_mms(vq0, 0, (2, 3))
        vproj_mms(vq1, 1, (0, 1))
        vproj_mms(vq1, 1, (2, 3))
        nc.sync.dma_start(out=wo_sb[:], in_=wov)
        kc1 = qkproj_dma(kTv, 1)
        kc2 = qkproj_dma(kTv, 2)
        vq2 = vproj_dma(2)
        vq3 = vproj_dma(3)

        # ---- attention pair 0 with inserted projection bursts ----
        pvt0 = [
            psa.tile([128, 1024], F32, tag="pvt", bufs=2, name=f"pvt0_{qh}")
            for qh in range(NQH)
        ]
        kc3 = None
        prev = None
        for kb in range(NKB):
            cur = attn_scores(0, kb)
            if prev is not None:
                attn_pv(pvt0, prev)
            prev = cur
            if kb == 0:
                qkproj_mms(kc1, wk_sb, bk_sb, kpT_sb, 1, (0,))
            elif kb == 1:
                qkproj_mms(kc1, wk_sb, bk_sb, kpT_sb, 1, (1,))
            elif kb == 2:
                vproj_mms(vq2, 2, (0, 1))
                kc3 = qkproj_dma(kTv, 3)
            elif kb == 3:
                vproj_mms(vq2, 2, (2, 3))
            elif kb == 4:
                qkproj_mms(kc2, wk_sb, bk_sb, kpT_sb, 2, (0,))
            elif kb == 5:
                qkproj_mms(kc2, wk_sb, bk_sb, kpT_sb, 2, (1,))
            elif kb == 6:
                vproj_mms(vq3, 3, (0, 1))
            elif kb == 7:
                vproj_mms(vq3, 3, (2, 3))
            elif kb == 8:
                qkproj_mms(kc3, wk_sb, bk_sb, kpT_sb, 3, (0,))
            elif kb == 9:
                qkproj_mms(kc3, wk_sb, bk_sb, kpT_sb, 3, (1,))
        attn_pv(pvt0, prev)
        for qh in range(NQH):
            nc.vector.tensor_copy(
                hcT_sb[:, 0, qh * 1024 : qh * 1024 + 1024], pvt0[qh][:]
            )

        # ---- attention pair 1; pair-0 O-projection rides along ----
        pvt1 = [
            psa.tile([128, 1024], F32, tag="pvt", bufs=2, name=f"pvt1_{qh}")
            for qh in range(NQH)
        ]
        prev = None
        for kb in range(NKB):
            cur = attn_scores(1, kb)
            if prev is not None:
                attn_pv(pvt1, prev)
            prev = cur
            oproj_tt(0, kb)
        attn_pv(pvt1, prev)
        for qh in range(NQH):
            nc.scalar.copy(
                hcT_sb[:, 1, qh * 1024 : qh * 1024 + 1024], pvt1[qh][:]
            )

        # ---- pair-1 O-projection tail (alternate drain engines) ----
        for tt in range(NTT):
            oproj_tt(1, tt, on_scalar=(tt % 2 == 0))

    nc.compile()
    return nc


# ---------------- host-side shard / unshard ----------------

B = 2

_NC_CACHE = {}


def _get_nc():
    if "nc" not in _NC_CACHE:
        _NC_CACHE["nc"] = build_kernel()
    return _NC_CACHE["nc"]


def make_in_maps(q, k, v, Wq, bq, Wk, bk, Wv, bv, Wo, bo):
    bf = ml_dtypes.bfloat16
    maps = []
    for c in range(8):
        b = c // 4
        hc = c % 4
        cols = slice(256 * hc, 256 * hc + 256)
        vt = (
            v[b]
            .astype(bf)
            .reshape(4, 512, NK, 128)
            .transpose(0, 3, 2, 1)
        )  # [quad, p(d-in-chunk), kk, t]
        maps.append({
            "qT": np.ascontiguousarray(
                q[b].astype(bf).reshape(NTC, 512, NK, 128).transpose(0, 3, 2, 1)
            ),
            "kT": np.ascontiguousarray(
                k[b].astype(bf).reshape(NTC, 512, NK, 128).transpose(0, 3, 2, 1)
            ),
            "vT": np.ascontiguousarray(vt),
            "wq": np.ascontiguousarray(Wq[:, cols].astype(bf)),
            "wk": np.ascontiguousarray(Wk[:, cols].astype(bf)),
            "wv": np.ascontiguousarray(Wv[:, cols].astype(bf)),
            "wo": np.ascontiguousarray(Wo[cols, :].astype(bf)),
            "bq": np.ascontiguousarray(
                bq[cols].reshape(NPAIR, 128, 1).astype(np.float32)
            ),
            "bk": np.ascontiguousarray(
                bk[cols].reshape(NPAIR, 128, 1).astype(np.float32)
            ),
            "bv": np.ascontiguousarray(bv[cols].astype(np.float32)),
        })
    return maps


def kernel(q, k, v, Wq, bq, Wk, bk, Wv, bv, Wo, bo):
    q = np.asarray(q, dtype=np.float32)
    k = np.asarray(k, dtype=np.float32)
    v = np.asarray(v, dtype=np.float32)
    Wq = np.asarray(Wq, dtype=np.float32)
    Wk = np.asarray(Wk, dtype=np.float32)
    Wv = np.asarray(Wv, dtype=np.float32)
    Wo = np.asarray(Wo, dtype=np.float32)
    bq = np.asarray(bq, dtype=np.float32)
    bk = np.asarray(bk, dtype=np.float32)
    bv = np.asarray(bv, dtype=np.float32)
    bo = np.asarray(bo, dtype=np.float32)

    nc = _get_nc()
    maps = make_in_maps(q, k, v, Wq, bq, Wk, bk, Wv, bv, Wo, bo)
    res = run_bass_kernel_spmd(nc, maps, core_ids=list(range(8)))

    outs = []
    for b in range(B):
        acc = np.zeros((S, D), dtype=np.float32)
        for hc in range(4):
            r = res.results[b * 4 + hc]
            acc += r["out0"].astype(np.float32)
            acc += r["out1"].astype(np.float32)
        acc += bo[None, :]
        outs.append(acc)
    return np.stack(outs, axis=0)


# revision 14
# speedup vs baseline: 1.1866x; 1.1255x over previous
"""Bass/Tile kernel for nn_MultiHeadAttention (B=2, S=2048, D=1024, H=16).

Sharding: 8 cores = 2 (batch) x 4 (head-chunks of 4 heads).

v6 design (baseline v1 ~307us):
  - All matmul paths in BF16 (inputs, weights, qp/kp/vp/at/hc): bf16
    LDWEIGHTS (125ns) hide under N=512 streams; input DMA halves.
    Host-side full-datapath sim: rel err 0.0112 vs the 2e-2 gate.
  - kb-inner attention with PV accumulated in PSUM across all 16
    k-blocks (start/stop) -> no DVE accumulation chain.
  - PV matmuls software-pipelined one kb behind the score/exp stream so
    the in-order PE queue never head-of-line blocks on the z-chain.
  - PSUM: 4 banks pvt (2x[128,1024] fp32, col-packed heads) + 4 banks
    score rotation (2x[128,1024]).  K-proj chunks 1-3, V-proj tiles
    4-15 and pair-0's O-projection ride the score rotation as short
    MM-only bursts (all DMA prefetched into dedicated buffers).
  - O-projection split per head-pair into out0/out1 bf16 partials
    (summed on host with bo): pair-0's O-proj + DMA runs during
    pair-1's attention; only pair-1's is tail.
"""

import sys

sys.path.insert(0, "/opt/trn_rl_repo")

from contextlib import ExitStack

import numpy as np
import ml_dtypes

import concourse.bass as bass
import concourse.mybir as mybir
import concourse.tile as tile
from concourse import bacc
from concourse.bass_utils import run_bass_kernel_spmd

BF16 = mybir.dt.bfloat16
F32 = mybir.dt.float32
AF = mybir.ActivationFunctionType
ALU = mybir.AluOpType

D = 1024
NK = 8  # k-tiles over D
DOUT = 256  # per-core head dims (4 heads)
NPAIR = 2  # pairs of heads (128 dout each)
HD = 64
S = 2048
NKB = S // 128  # 16 k-token blocks
NQH = 2  # q halves of 1024
NTC = S // 512  # 4 projection token chunks
NTT = S // 128  # 16 token tiles


def build_kernel():
    nc = bacc.Bacc("TRN2", target_bir_lowering=False, debug=False)

    qT = nc.dram_tensor("qT", [D, S], BF16, kind="ExternalInput")
    kT = nc.dram_tensor("kT", [D, S], BF16, kind="ExternalInput")
    vT = nc.dram_tensor("vT", [4, 128, NK, 512], BF16, kind="ExternalInput")
    wq = nc.dram_tensor("wq", [D, DOUT], BF16, kind="ExternalInput")
    wk = nc.dram_tensor("wk", [D, DOUT], BF16, kind="ExternalInput")
    wv = nc.dram_tensor("wv", [D, DOUT], BF16, kind="ExternalInput")
    wo = nc.dram_tensor("wo", [DOUT, D], BF16, kind="ExternalInput")
    bq = nc.dram_tensor("bq", [NPAIR, 128, 1], F32, kind="ExternalInput")
    bk = nc.dram_tensor("bk", [NPAIR, 128, 1], F32, kind="ExternalInput")
    bv = nc.dram_tensor("bv", [DOUT], F32, kind="ExternalInput")
    out0 = nc.dram_tensor("out0", [S, D], BF16, kind="ExternalOutput")
    out1 = nc.dram_tensor("out1", [S, D], BF16, kind="ExternalOutput")

    qTv = qT.ap().rearrange("(t p) s -> p t s", p=128)  # [128, NK, S]
    kTv = kT.ap().rearrange("(t p) s -> p t s", p=128)
    vTv = vT.ap()  # [4, 128, NK, 512] quad-major
    wqv = wq.ap().rearrange("(t p) m -> p t m", p=128)  # [128, 8, 256]
    wkv = wk.ap().rearrange("(t p) m -> p t m", p=128)
    wvv = wv.ap().rearrange("(t p) m -> p t m", p=128)
    wov = wo.ap().rearrange("(t p) m -> p t m", p=128)  # [128, 2, 1024]
    bqv = bq.ap().rearrange("a p o -> p a o")  # [128, 2, 1]
    bkv = bk.ap().rearrange("a p o -> p a o")
    outv = [
        out0.ap().rearrange("(tb two p) m -> tb p two m", two=2, p=128),
        out1.ap().rearrange("(tb two p) m -> tb p two m", two=2, p=128),
    ]  # [8, 128, 2, 1024]

    bv_bcast_ap = bass.AP(tensor=bv.ap().tensor, offset=0, ap=[[0, 128], [1, DOUT]])

    with tile.TileContext(nc) as tc, ExitStack() as ctx:
        sb = ctx.enter_context(tc.tile_pool(name="sb", bufs=1))
        xtp = ctx.enter_context(tc.tile_pool(name="xt_stream", bufs=1))
        vtp = ctx.enter_context(tc.tile_pool(name="vt_stream", bufs=1))
        atp = ctx.enter_context(tc.tile_pool(name="at_pool", bufs=1))
        zp = ctx.enter_context(tc.tile_pool(name="z_pool", bufs=1))
        osb = ctx.enter_context(tc.tile_pool(name="o_sb", bufs=1))
        psa = ctx.enter_context(tc.tile_pool(name="ps_all", bufs=1, space="PSUM"))

        # ---- resident weights/biases (all small, bf16) ----
        wq_sb = sb.tile([128, NK, DOUT], BF16, tag="wq")
        wk_sb = sb.tile([128, NK, DOUT], BF16, tag="wk")
        wv_sb = sb.tile([128, NK, DOUT], BF16, tag="wv")
        wo_sb = sb.tile([128, NPAIR, D], BF16, tag="wo")
        bq_sb = sb.tile([128, NPAIR, 1], F32, tag="bq")
        bk_sb = sb.tile([128, NPAIR, 1], F32, tag="bk")
        bv_sb = sb.tile([128, DOUT], F32, tag="bv")
        nc.sync.dma_start(out=wq_sb[:], in_=wqv)
        nc.sync.dma_start(out=wk_sb[:], in_=wkv)
        nc.sync.dma_start(out=wv_sb[:], in_=wvv)
        nc.sync.dma_start(out=bq_sb[:], in_=bqv)
        nc.sync.dma_start(out=bk_sb[:], in_=bkv)
        nc.sync.dma_start(out=bv_sb[:], in_=bv_bcast_ap)

        # ---- projection outputs (resident bf16) ----
        qpT_sb = sb.tile([128, NPAIR, S], BF16, tag="qpT")
        kpT_sb = sb.tile([128, NPAIR, S], BF16, tag="kpT")
        vp_sb = sb.tile([128, NTT, DOUT], BF16, tag="vp")
        hcT_sb = sb.tile([128, NPAIR, S], BF16, tag="hcT")

        def qkproj_dma(XTv, tci):
            tsl = slice(tci * 512, tci * 512 + 512)
            xt = xtp.tile([128, NK, 512], BF16, tag="xt", bufs=5, name="xt")
            nc.sync.dma_start(out=xt[:], in_=XTv[:, :, tsl])
            return xt

        def qkproj_mms(xts, W_sb, b_sb, XPT, tci, pairs):
            tsl = slice(tci * 512, tci * 512 + 512)
            ps = psa.tile([128, 1024], F32, tag="sc", bufs=2, name="pj")
            for kk in range(NK):
                for p in pairs:
                    nc.tensor.matmul(
                        ps[:, p * 512 : p * 512 + 512],
                        lhsT=W_sb[:, kk, p * 128 : p * 128 + 128],
                        rhs=xts[:, kk, :],
                        start=(kk == 0),
                        stop=(kk == NK - 1),
                        skip_group_check=True,
                    )
            for p in pairs:
                nc.vector.tensor_scalar_add(
                    XPT[:, p, tsl], ps[:, p * 512 : p * 512 + 512], b_sb[:, p, :]
                )

        def qkproj_chunk(XTv, W_sb, b_sb, XPT, tci):
            qkproj_mms(qkproj_dma(XTv, tci), W_sb, b_sb, XPT, tci, (0, 1))

        def vproj_dma(tq):
            vt = vtp.tile([128, NK, 512], BF16, tag="vt", bufs=3, name="vt")
            nc.sync.dma_start(out=vt[:], in_=vTv[tq])
            return vt

        def vproj_mms(vts, tq, idxs):
            # V projection for the given tile indices within quad tq
            ps = psa.tile([128, 1024], F32, tag="sc", bufs=2, name="vj")
            for j, i in enumerate(idxs):
                for kk in range(NK):
                    nc.tensor.matmul(
                        ps[:, j * 256 : j * 256 + 256],
                        lhsT=vts[:, kk, i * 128 : i * 128 + 128],
                        rhs=wv_sb[:, kk, :],
                        start=(kk == 0),
                        stop=(kk == NK - 1),
                        skip_group_check=True,
                    )
            for j, i in enumerate(idxs):
                nc.vector.scalar_tensor_tensor(
                    out=vp_sb[:, tq * 4 + i, :],
                    in0=ps[:, j * 256 : j * 256 + 256],
                    scalar=1.0,
                    in1=bv_sb[:],
                    op0=ALU.mult,
                    op1=ALU.add,
                )

        ost_cur = [None]

        def oproj_tt(p, tt, on_scalar=False):
            # fills half of a [128,2048] staging tile; DMA fires per tt pair
            ps = psa.tile([128, 1024], F32, tag="sc", bufs=2, name="oj")
            for dc in range(2):
                nc.tensor.matmul(
                    ps[:, dc * 512 : dc * 512 + 512],
                    lhsT=hcT_sb[:, p, tt * 128 : tt * 128 + 128],
                    rhs=wo_sb[:, p, dc * 512 : dc * 512 + 512],
                    start=True,
                    stop=True,
                    skip_group_check=True,
                )
            if tt % 2 == 0:
                ost_cur[0] = osb.tile(
                    [128, 2, 1024], BF16, tag="ost", bufs=3, name="ost"
                )
            ost = ost_cur[0]
            half = ost[:, tt % 2, :]
            if on_scalar:
                nc.scalar.copy(half, ps[:])
            else:
                nc.vector.tensor_copy(half, ps[:])
            if tt % 2 == 1:
                nc.sync.dma_start(out=outv[p][tt // 2], in_=ost[:])

        def attn_scores(p, kb):
            # scores + exp + z-chain for one k-block; returns PV operands
            ksl = slice(kb * 128, kb * 128 + 128)
            at_t = {}
            z_t = {}
            for qh in range(NQH):
                scs = [
                    psa.tile([128, 1024], F32, tag="sc", bufs=2, name=f"sc{h}")
                    for h in range(2)
                ]
                for qq in range(2):
                    for h in range(2):
                        hsl = slice(h * 64, h * 64 + 64)
                        qsl = slice(
                            qh * 1024 + qq * 512, qh * 1024 + qq * 512 + 512
                        )
                        nc.tensor.matmul(
                            scs[h][:, qq * 512 : qq * 512 + 512],
                            lhsT=kpT_sb[hsl, p, ksl],
                            rhs=qpT_sb[hsl, p, qsl],
                            start=True,
                            stop=True,
                            skip_group_check=True,
                        )
                for h in range(2):
                    at = atp.tile(
                        [128, 1024], BF16, tag=f"at{h}{qh}", bufs=6, name="at"
                    )
                    z = zp.tile([128, 1], F32, tag=f"z{h}{qh}", bufs=6, name="z")
                    nc.scalar.activation(
                        out=at[:], in_=scs[h][:], func=AF.Exp, accum_out=z[:]
                    )
                    at_t[(h, qh)] = at
                    z_t[(h, qh)] = z
            vhss = {}
            for h in range(2):
                zs = zp.tile([128, 1], F32, tag=f"zs{h}", bufs=4, name="zs")
                nc.vector.tensor_add(zs[:], z_t[(h, 0)][:], z_t[(h, 1)][:])
                rz = zp.tile([128, 1], F32, tag=f"rz{h}", bufs=4, name="rz")
                nc.vector.reciprocal(rz[:], zs[:])
                vhs = zp.tile([128, HD], BF16, tag=f"vh{h}", bufs=4, name="vhs")
                nc.vector.tensor_scalar_mul(
                    vhs[:],
                    vp_sb[:, kb, p * 128 + h * 64 : p * 128 + h * 64 + 64],
                    rz[:],
                )
                vhss[h] = vhs
            return (kb, at_t, vhss)

        def attn_pv(pvt, state):
            # PV matmuls for a previous k-block (h-adjacent: col groups
            # run concurrently); accumulates into resident pvt tiles
            kb, at_t, vhss = state
            for qh in range(NQH):
                for qq in range(2):
                    for h in range(2):
                        nc.tensor.matmul(
                            pvt[qh][
                                h * 64 : h * 64 + 64, qq * 512 : qq * 512 + 512
                            ],
                            lhsT=vhss[h][:],
                            rhs=at_t[(h, qh)][:, qq * 512 : qq * 512 + 512],
                            start=(kb == 0),
                            stop=(kb == NKB - 1),
                            tile_position=(0, h * 64),
                            skip_group_check=True,
                        )

        # ---- pre-phase: Q-proj (all), K-proj chunk 0, V-proj quad 0 ----
        for tci in range(NTC):
            qkproj_chunk(qTv, wq_sb, bq_sb, qpT_sb, tci)
        qkproj_chunk(kTv, wk_sb, bk_sb, kpT_sb, 0)
        vq0 = vproj_dma(0)
        vproj_mms(vq0, 0, (0, 1))
        vproj_mms(vq0, 0, (2, 3))
        vq1 = vproj_dma(1)
        vproj_mms(vq1, 1, (0, 1))
        vproj_mms(vq1, 1, (2, 3))
        nc.sync.dma_start(out=wo_sb[:], in_=wov)
        kc1 = qkproj_dma(kTv, 1)
        kc2 = qkproj_dma(kTv, 2)

        # ---- attention pair 0 with inserted projection bursts ----
        pvt0 = [
            psa.tile([128, 1024], F32, tag="pvt", bufs=2, name=f"pvt0_{qh}")
            for qh in range(NQH)
        ]
        kc3 = None
        vq2 = vq3 = None
        prev = None
        for kb in range(NKB):
            cur = attn_scores(0, kb)
            if prev is not None:
                attn_pv(pvt0, prev)
            prev = cur
            if kb == 0:
                qkproj_mms(kc1, wk_sb, bk_sb, kpT_sb, 1, (0,))
                vq2 = vproj_dma(2)
            elif kb == 1:
                qkproj_mms(kc1, wk_sb, bk_sb, kpT_sb, 1, (1,))
            elif kb == 2:
                vproj_mms(vq2, 2, (0, 1))
                kc3 = qkproj_dma(kTv, 3)
            elif kb == 3:
                vproj_mms(vq2, 2, (2, 3))
                vq3 = vproj_dma(3)
            elif kb == 4:
                qkproj_mms(kc2, wk_sb, bk_sb, kpT_sb, 2, (0,))
            elif kb == 5:
                qkproj_mms(kc2, wk_sb, bk_sb, kpT_sb, 2, (1,))
            elif kb == 6:
                vproj_mms(vq3, 3, (0, 1))
            elif kb == 7:
                vproj_mms(vq3, 3, (2, 3))
            elif kb == 8:
                qkproj_mms(kc3, wk_sb, bk_sb, kpT_sb, 3, (0,))
            elif kb == 9:
                qkproj_mms(kc3, wk_sb, bk_sb, kpT_sb, 3, (1,))
        attn_pv(pvt0, prev)
        for qh in range(NQH):
            nc.vector.tensor_copy(
                hcT_sb[:, 0, qh * 1024 : qh * 1024 + 1024], pvt0[qh][:]
            )

        # ---- attention pair 1; pair-0 O-projection rides along ----
        pvt1 = [
            psa.tile([128, 1024], F32, tag="pvt", bufs=2, name=f"pvt1_{qh}")
            for qh in range(NQH)
        ]
        prev = None
        for kb in range(NKB):
            cur = attn_scores(1, kb)
            if prev is not None:
                attn_pv(pvt1, prev)
            prev = cur
            oproj_tt(0, kb)
        attn_pv(pvt1, prev)
        for qh in range(NQH):
            nc.scalar.copy(
                hcT_sb[:, 1, qh * 1024 : qh * 1024 + 1024], pvt1[qh][:]
            )

        # ---- pair-1 O-projection tail (alternate drain engines) ----
        for tt in range(NTT):
            oproj_tt(1, tt, on_scalar=(tt % 2 == 0))

    nc.compile()
    return nc


# ---------------- host-side shard / unshard ----------------

B = 2

_NC_CACHE = {}


def _get_nc():
    if "nc" not in _NC_CACHE:
        _NC_CACHE["nc"] = build_kernel()
    return _NC_CACHE["nc"]


def make_in_maps(q, k, v, Wq, bq, Wk, bk, Wv, bv, Wo, bo):
    bf = ml_dtypes.bfloat16
    maps = []
    for c in range(8):
        b = c // 4
        hc = c % 4
        cols = slice(256 * hc, 256 * hc + 256)
        vt = (
            v[b]
            .astype(bf)
            .reshape(4, 512, NK, 128)
            .transpose(0, 3, 2, 1)
        )  # [quad, p(d-in-chunk), kk, t]
        maps.append({
            "qT": np.ascontiguousarray(q[b].T.astype(bf)),
            "kT": np.ascontiguousarray(k[b].T.astype(bf)),
            "vT": np.ascontiguousarray(vt),
            "wq": np.ascontiguousarray(Wq[:, cols].astype(bf)),
            "wk": np.ascontiguousarray(Wk[:, cols].astype(bf)),
            "wv": np.ascontiguousarray(Wv[:, cols].astype(bf)),
            "wo": np.ascontiguousarray(Wo[cols, :].astype(bf)),
            "bq": np.ascontiguousarray(
                bq[cols].reshape(NPAIR, 128, 1).astype(np.float32)
            ),
            "bk": np.ascontiguousarray(
                bk[cols].reshape(NPAIR, 128, 1).astype(np.float32)
            ),
            "bv": np.ascontiguousarray(bv[cols].astype(np.float32)),
        })
    return maps


def kernel(q, k, v, Wq, bq, Wk, bk, Wv, bv, Wo, bo):
    q = np.asarray(q, dtype=np.float32)
    k = np.asarray(k, dtype=np.float32)
    v = np.asarray(v, dtype=np.float32)
    Wq = np.asarray(Wq, dtype=np.float32)
    Wk = np.asarray(Wk, dtype=np.float32)
    Wv = np.asarray(Wv, dtype=np.float32)
    Wo = np.asarray(Wo, dtype=np.float32)
    bq = np.asarray(bq, dtype=np.float32)
    bk = np.asarray(bk, dtype=np.float32)
    bv = np.asarray(bv, dtype=np.float32)
    bo = np.asarray(bo, dtype=np.float32)

    nc = _get_nc()
    maps = make_in_maps(q, k, v, Wq, bq, Wk, bk, Wv, bv, Wo, bo)
    res = run_bass_kernel_spmd(nc, maps, core_ids=list(range(8)))

    outs = []
    for b in range(B):
        acc = np.zeros((S, D), dtype=np.float32)
        for hc in range(4):
            r = res.results[b * 4 + hc]
            acc += r["out0"].astype(np.float32)
            acc += r["out1"].astype(np.float32)
        acc += bo[None, :]
        outs.append(acc)
    return np.stack(outs, axis=0)
